# revision 6
# baseline (speedup 1.0000x reference)
"""DistributedGCN on 8 TRN2 NeuronCores — v3: pair-compacted SBUF scatter.

Layout: shard SH=12500 -> SHP=12544=98*128; node d <-> (p=d%128, gd=d//128);
token(d) = d: slot=gd, parity=gd&1, col=gd>>1 in acc[parity] [128, 49*64].
Tables (x and layer acts) hold activations prescaled by dinv; pad rows zero.

Edge phase per layer, per bucket b (src core), sections:
  pair rounds r=0,1: for each dst with c_b(d) > 2r: two slots (occ_b 2r, 2r+1;
    missing second -> zero-row pad). Gather [firsts|seconds] into msg tile,
    DVE add halves -> pair tile, dma_scatter_add (SBUF dst, unique tokens).
  raw rounds r'=0..: edges with occ_b = 4+r' scattered raw (unique per call).
All scatter calls WAW-chain on acc (safe). s = acc * dinv. Then per-group
PE transpose + matmul -> node-major z, BN via reduce+ones-matmul+AllReduce,
apply + relu + dinv, AllGather next table. Layer3: +b3 -> y.
"""

from contextlib import ExitStack

import numpy as np

import concourse.bass as bass
import concourse.tile as tile
from concourse import bacc, mybir
from concourse.bass_utils import run_bass_kernel_spmd

F32 = mybir.dt.float32
I16 = mybir.dt.int16

MAX_PAIR_HALF = 512         # slots per half-block (walrus ring limit ~1024 desc/call)


def _wrap16(vals, S, pad=-1):
    lin = np.full(16 * S, pad, np.int16)
    lin[:len(vals)] = vals
    return np.tile(lin.reshape(S, 16).T.copy(), (8, 1))


def host_prep(x, edge_index, n_cores=8):
    N, D = x.shape
    SH = N // n_cores
    assert SH * n_cores == N
    GD = (SH + 127) // 128
    assert GD % 2 == 0, "GD must be even for parity split"
    SHP = GD * 128
    HC = GD // 2
    NPAD = SHP - SH
    assert NPAD > 0, "need pad node slots for trash tokens"

    src = np.asarray(edge_index[0], dtype=np.int64)
    dst = np.asarray(edge_index[1], dtype=np.int64)
    deg = np.bincount(dst, minlength=N).astype(np.float64) + 1.0
    dinv = (1.0 / np.sqrt(deg)).astype(np.float32)
    own = dst // SH

    # per-core edge arrays sorted by (dst, bucket); occ_b = rank within (d,b)
    cores = []
    for c in range(n_cores):
        m = own == c
        es = np.concatenate([np.arange(c * SH, (c + 1) * SH, dtype=np.int64), src[m]])
        ed = np.concatenate([np.arange(c * SH, (c + 1) * SH, dtype=np.int64), dst[m]])
        b = es // SH
        o = np.lexsort((b, ed))
        es, ed, b = es[o], ed[o], b[o]
        d_loc = ed - c * SH
        changeb = np.empty(len(ed), bool)
        changeb[0] = True
        changeb[1:] = (ed[1:] != ed[:-1]) | (b[1:] != b[:-1])
        runb = np.maximum.accumulate(np.where(changeb, np.arange(len(ed)), 0))
        occ_b = np.arange(len(ed)) - runb
        g_loc = es - b * SH
        cores.append((d_loc, b, occ_b, g_loc))

    # section sizes: for (b, "pair", r): count dsts with c_b > 2r ; for
    # (b, "raw", r'): count edges with occ_b == 4 + r'
    def sec_counts(c):
        d_loc, b, occ_b, g_loc = cores[c]
        pair_cnt = np.zeros((n_cores, 2), np.int64)
        raw_cnt = {}
        for bb in range(n_cores):
            mm = b == bb
            ob = occ_b[mm]
            pair_cnt[bb, 0] = int((ob == 0).sum())
            pair_cnt[bb, 1] = int((ob == 2).sum())
            mx = int(ob.max()) if len(ob) else -1
            for r in range(max(0, mx - 3)):
                raw_cnt[(bb, r)] = int((ob == 4 + r).sum())
        return pair_cnt, raw_cnt

    allp, allr = [], []
    for c in range(n_cores):
        p_, r_ = sec_counts(c)
        allp.append(p_)
        allr.append(r_)
    pair_max = np.stack(allp).max(axis=0)          # [bucket, round]
    raw_keys = sorted(set(k for r_ in allr for k in r_))
    raw_max = {k: max(r_.get(k, 0) for r_ in allr) for k in raw_keys}

    # static plan: chunks; each chunk = list of sections
    # section tuple: (kind, bucket, round, half_slots or slots)
    # chunk constraint: total gather slots <= ~11264
    plan = []               # list of chunks; chunk = list of sections
    for bb in range(n_cores):
        secs = []
        for r in (0, 1):
            n = int(pair_max[bb, r])
            if n == 0:
                continue
            nsub = max(1, (n + MAX_PAIR_HALF - 1) // MAX_PAIR_HALF)
            per = (n + nsub - 1) // nsub
            per = ((per + 127) // 128) * 128
            for s in range(nsub):
                secs.append(("pair", bb, r, s, per))
        for (bbb, r) in raw_keys:
            if bbb != bb:
                continue
            n = raw_max[(bb, r)]
            if n == 0:
                continue
            nsub = max(1, (n + 1023) // 1024)
            per = (n + nsub - 1) // nsub
            per = ((per + 127) // 128) * 128
            for s in range(nsub):
                secs.append(("raw", bb, r, s, per))
        # greedy-pack sections into chunks by gather slots
        cur, cur_slots = [], 0
        for sec in secs:
            slots = sec[4] * (2 if sec[0] == "pair" else 1)
            if cur and cur_slots + slots > 2 * MAX_PAIR_HALF:
                plan.append(cur)
                cur, cur_slots = [], 0
            cur.append(sec)
            cur_slots += slots
        if cur:
            plan.append(cur)

    # gather/scatter idx arrays per core following plan
    per_core = []
    for c in range(n_cores):
        d_loc, b, occ_b, g_loc = cores[c]
        gi_cols, si_cols = [], []
        for chunk in plan:
            for (kind, bb, r, s, per) in chunk:
                mm = b == bb
                if kind == "pair":
                    m1 = mm & (occ_b == 2 * r)
                    m2 = mm & (occ_b == 2 * r + 1)
                    d1, gl1 = d_loc[m1], g_loc[m1]
                    d2, gl2 = d_loc[m2], g_loc[m2]
                    lo, hi = s * per, (s + 1) * per
                    d1s = d1[lo:hi] if lo < len(d1) else d1[:0]
                    g1s = gl1[lo:hi] if lo < len(gl1) else gl1[:0]
                    # seconds aligned to firsts by dst
                    pos = {int(dd): i for i, dd in enumerate(d1s)}
                    g2s = np.full(len(d1s), -1, np.int64)
                    for dd, gg in zip(d2, gl2):
                        i = pos.get(int(dd))
                        if i is not None:
                            g2s[i] = gg
                    # pads: firsts-pad -> zero rows; seconds missing -> zero rows
                    zrow = SH + (np.arange(per) % NPAD)
                    gfirst = np.concatenate([g1s, zrow[len(d1s):per]])
                    gsec = np.where(g2s >= 0, g2s, zrow[:len(d1s)])
                    gsec = np.concatenate([gsec, zrow[len(d1s):per]])
                    S = per // 16
                    gi_cols.append(_wrap16(gfirst.astype(np.int16), S, pad=0))
                    gi_cols.append(_wrap16(gsec.astype(np.int16), S, pad=0))
                    tok = np.concatenate([
                        d1s, SH + (np.arange(per - len(d1s)) % NPAD)])
                    si_cols.append(_wrap16(tok.astype(np.int16), S))
                else:
                    m1 = mm & (occ_b == 4 + r)
                    d1, gl1 = d_loc[m1], g_loc[m1]
                    lo, hi = s * per, (s + 1) * per
                    d1 = d1[lo:hi] if lo < len(d1) else d1[:0]
                    gl1 = gl1[lo:hi] if lo < len(gl1) else gl1[:0]
                    zrow = SH + (np.arange(per) % NPAD)
                    gfull = np.concatenate([gl1, zrow[len(d1):per]])
                    tok = np.concatenate([
                        d1, SH + (np.arange(per - len(d1)) % NPAD)])
                    S = per // 16
                    gi_cols.append(_wrap16(gfull.astype(np.int16), S, pad=0))
                    si_cols.append(_wrap16(tok.astype(np.int16), S))
        dinv_par = np.zeros((2, 128, HC), np.float32)
        dl = np.arange(SH)
        dinv_par[(dl // 128) & 1, dl % 128, (dl // 128) >> 1] = dinv[c * SH:(c + 1) * SH]
        per_core.append({
            "gi": np.concatenate(gi_cols, axis=1),
            "si": np.concatenate(si_cols, axis=1),
            "dinv_e": dinv_par[0], "dinv_o": dinv_par[1],
        })

    xt = np.zeros((n_cores * SHP, D), dtype=np.float32)
    rows = (np.arange(N) // SH) * SHP + (np.arange(N) % SH)
    xt[rows] = x * dinv[:, None]

    S_gi = per_core[0]["gi"].shape[1]
    S_si = per_core[0]["si"].shape[1]
    meta = {
        "N": N, "D": D, "n_cores": n_cores, "SH": SH, "SHP": SHP, "GD": GD,
        "HC": HC, "plan": plan, "S_gi": S_gi, "S_si": S_si,
    }
    return meta, per_core, xt


def build_kernel(meta):
    D = meta["D"]
    C = meta["n_cores"]
    SHP = meta["SHP"]
    GD = meta["GD"]
    HC = meta["HC"]
    plan = meta["plan"]
    inv_n = 1.0 / meta["N"]
    EPS = 1e-5
    S_gi, S_si = meta["S_gi"], meta["S_si"]

    nc = bacc.Bacc("TRN2", target_bir_lowering=False, debug=False, num_devices=C)

    xt = nc.dram_tensor("xt", [C * SHP, D], F32, kind="ExternalInput").ap()
    gi = nc.dram_tensor("gi", [128, S_gi], I16, kind="ExternalInput").ap()
    si = nc.dram_tensor("si", [128, S_si], I16, kind="ExternalInput").ap()
    dinv_e_io = nc.dram_tensor("dinv_e", [128, HC], F32, kind="ExternalInput").ap()
    dinv_o_io = nc.dram_tensor("dinv_o", [128, HC], F32, kind="ExternalInput").ap()
    Ws = [nc.dram_tensor(f"W{i}", [D, D], F32, kind="ExternalInput").ap() for i in (1, 2, 3)]
    gs = [nc.dram_tensor(f"g{i}", [1, D], F32, kind="ExternalInput").ap() for i in (1, 2)]
    bes = [nc.dram_tensor(f"be{i}", [1, D], F32, kind="ExternalInput").ap() for i in (1, 2)]
    b3 = nc.dram_tensor("b3", [1, D], F32, kind="ExternalInput").ap()
    y = nc.dram_tensor("y", [SHP, D], F32, kind="ExternalOutput").ap()

    RG = [list(range(C))]
    ag_in = [nc.dram_tensor(f"ag_in{l}", [SHP, D], F32).ap() for l in (2, 3)]
    tables = [xt]
    for l in (2, 3):
        tables.append(nc.dram_tensor(f"table{l}", [C * SHP, D], F32, addr_space="Shared").ap())
    ar_in = [nc.dram_tensor(f"ar_in{l}", [D, 2], F32).ap() for l in (1, 2)]
    ar_out = [nc.dram_tensor(f"ar_out{l}", [D, 2], F32, addr_space="Shared").ap() for l in (1, 2)]

    with tile.TileContext(nc) as tc, ExitStack() as ctx:
        msg_p = ctx.enter_context(tc.tile_pool(name="msgs", bufs=3))
        pr_p = ctx.enter_context(tc.tile_pool(name="pairs", bufs=2))
        idx_p = ctx.enter_context(tc.tile_pool(name="idx", bufs=4))
        sfm_p = ctx.enter_context(tc.tile_pool(name="sfm", bufs=3))
        ps_p = ctx.enter_context(tc.tile_pool(name="psum", bufs=2, space="PSUM"))
        ps1_p = ctx.enter_context(tc.tile_pool(name="psum1", bufs=1, space="PSUM"))
        sm_p = ctx.enter_context(tc.tile_pool(name="small", bufs=4))

        dinv_t = [nc.alloc_sbuf_tensor(f"dinv_sb{p}", [128, HC], F32).ap() for p in range(2)]
        nc.sync.dma_start(dinv_t[0][:], dinv_e_io[:])
        nc.sync.dma_start(dinv_t[1][:], dinv_o_io[:])
        W_t = []
        for i in range(3):
            t = nc.alloc_sbuf_tensor(f"W{i}_sb", [D, D], F32).ap()
            nc.sync.dma_start(t[:], Ws[i][:])
            W_t.append(t)
        ones_t = nc.alloc_sbuf_tensor("ones_sb", [128, 1], F32).ap()
        nc.gpsimd.memset(ones_t[:], 1.0)
        ones_row = nc.alloc_sbuf_tensor("ones_row_sb", [1, 128], F32).ap()
        nc.gpsimd.memset(ones_row[:], 1.0)
        ident_t = nc.alloc_sbuf_tensor("ident_sb", [128, 128], F32).ap()
        from concourse.masks import make_identity
        make_identity(nc, ident_t[:])

        acc = [nc.alloc_sbuf_tensor(f"acc{p}", [128, HC * D], F32).ap() for p in range(2)]
        zs = nc.alloc_sbuf_tensor("zs_sb", [128, GD * D], F32).ap()
        sq_sb = nc.alloc_sbuf_tensor("sq_sb", [128, GD * D], F32).ap()

        # max section sizes for pools
        max_chunk_slots = max(
            sum(sec[4] * (2 if sec[0] == "pair" else 1) for sec in chunk)
            for chunk in plan)
        max_pair = max((sec[4] for chunk in plan for sec in chunk
                        if sec[0] == "pair"), default=128)
        max_sec = max(sec[4] for chunk in plan for sec in chunk)

        for layer in (1, 2, 3):
            table = tables[layer - 1]
            nc.gpsimd.memset(acc[0][:], 0.0)
            nc.gpsimd.memset(acc[1][:], 0.0)
            gk = sk = 0
            for chunk in plan:
                slots = sum(sec[4] * (2 if sec[0] == "pair" else 1) for sec in chunk)
                Sg = slots // 16
                git = idx_p.tile([128, max_chunk_slots // 16], I16, tag="gi")
                nc.sync.dma_start(git[:, :Sg], gi[:, gk:gk + Sg])
                mt = msg_p.tile([128, (max_chunk_slots // 128) * D], F32, tag="msg")
                # one gather per chunk (single bucket per chunk by construction)
                bb = chunk[0][1]
                nc.gpsimd.dma_gather(
                    out_ap=mt[:, :(slots // 128) * D].rearrange("p (t e) -> p t e", e=D),
                    in_ap=table[bb * SHP:(bb + 1) * SHP, :],
                    idxs_ap=git[:, :Sg],
                    num_idxs=slots, num_idxs_reg=slots, elem_size=D,
                )
                gk += Sg
                off = 0   # column offset (in 128-slot cols) within msg tile
                for (kind, bb2, r, s, per) in chunk:
                    Tn = per // 128
                    Ss = per // 16
                    sit = idx_p.tile([128, max_sec // 16], I16, tag="si")
                    nc.sync.dma_start(sit[:, :Ss], si[:, sk:sk + Ss])
                    sk += Ss
                    if kind == "pair":
                        pt = pr_p.tile([128, (max_pair // 128) * D], F32, tag="pair")
                        nc.vector.tensor_tensor(
                            out=pt[:, :Tn * D],
                            in0=mt[:, off * D:(off + Tn) * D],
                            in1=mt[:, (off + Tn) * D:(off + 2 * Tn) * D],
                            op=mybir.AluOpType.add)
                        nc.gpsimd.dma_scatter_add(
                            acc[0][:], pt[:, :Tn * D].rearrange("p (t e) -> p t e", e=D),
                            sit[:, :Ss], per, per, D,
                            sbuf_tokens_per_rank=128, parity_reg=0,
                            out_ap_other=acc[1][:])
                        off += 2 * Tn
                    else:
                        nc.gpsimd.dma_scatter_add(
                            acc[0][:], mt[:, off * D:(off + Tn) * D].rearrange("p (t e) -> p t e", e=D),
                            sit[:, :Ss], per, per, D,
                            sbuf_tokens_per_rank=128, parity_reg=0,
                            out_ap_other=acc[1][:])
                        off += Tn
            # ---- s = acc * dinv (in place) ----
            for p in range(2):
                nc.vector.tensor_tensor(
                    out=acc[p][:].rearrange("p (g e) -> p g e", e=D),
                    in0=acc[p][:].rearrange("p (g e) -> p g e", e=D),
                    in1=dinv_t[p][:, :, None].to_broadcast([128, HC, D]),
                    op=mybir.AluOpType.mult)
            # ---- GEMM per group ----
            for g in range(GD):
                par, g2 = g & 1, g >> 1
                tp = ps_p.tile([D, 128], F32, space="PSUM", tag="tp")
                nc.tensor.transpose(
                    out=tp[:], in_=acc[par][:, g2 * D:(g2 + 1) * D],
                    identity=ident_t[:])
                sf = sfm_p.tile([D, 128], F32, tag="sf")
                nc.scalar.activation(sf[:], tp[:], mybir.ActivationFunctionType.Copy)
                zp = ps_p.tile([128, D], F32, space="PSUM", tag="zp")
                nc.tensor.matmul(zp[:], lhsT=sf[:], rhs=W_t[layer - 1][:],
                                 start=True, stop=True)
                nc.scalar.activation(zs[:, g * D:(g + 1) * D], zp[:],
                                     mybir.ActivationFunctionType.Copy)

            if layer < 3:
                gl, bel, arin, arout, agin, tnext = (
                    gs[layer - 1], bes[layer - 1], ar_in[layer - 1],
                    ar_out[layer - 1], ag_in[layer - 1], tables[layer])
                part = sm_p.tile([128, D], F32, tag="part")
                nc.vector.tensor_reduce(
                    out=part[:, :, None],
                    in_=zs[:].rearrange("p (g e) -> p e g", e=D),
                    axis=mybir.AxisListType.X, op=mybir.AluOpType.add)
                sq = sq_sb
                nc.vector.tensor_tensor(out=sq[:, :GD * D], in0=zs[:], in1=zs[:],
                                        op=mybir.AluOpType.mult)
                part2 = sm_p.tile([128, D], F32, tag="part2")
                nc.vector.tensor_reduce(
                    out=part2[:, :, None],
                    in_=sq[:, :GD * D].rearrange("p (g e) -> p e g", e=D),
                    axis=mybir.AxisListType.X, op=mybir.AluOpType.add)
                stp = ps1_p.tile([D, 2], F32, space="PSUM", tag="stats")
                nc.tensor.matmul(stp[:, 0:1], lhsT=part[:], rhs=ones_t[:],
                                 start=True, stop=True)
                nc.tensor.matmul(stp[:, 1:2], lhsT=part2[:], rhs=ones_t[:],
                                 start=True, stop=True)
                stats_sb = sm_p.tile([D, 2], F32, tag="stats_sb")
                nc.scalar.activation(stats_sb[:], stp[:],
                                     mybir.ActivationFunctionType.Copy)
                nc.sync.dma_start(arin[:], stats_sb[:])
                nc.gpsimd.collective_compute(
                    "AllReduce", mybir.AluOpType.add, ins=[arin[:]],
                    outs=[arout[:]], replica_groups=RG)
                stats_g = sm_p.tile([D, 2], F32, tag="stats_g")
                nc.sync.dma_start(stats_g[:], arout[:])
                invn1 = sm_p.tile([1, 1], F32, tag="invn1")
                nc.gpsimd.memset(invn1[:], inv_n)
                eps1 = sm_p.tile([1, 1], F32, tag="eps1")
                nc.gpsimd.memset(eps1[:], EPS)
                stt0 = ps1_p.tile([1, D], F32, space="PSUM", tag="stt")
                nc.tensor.transpose(out=stt0[:], in_=stats_g[:, 0:1],
                                    identity=ident_t[:D, :D])
                mean_r = sm_p.tile([1, D], F32, tag="mean_r")
                nc.scalar.activation(mean_r[:], stt0[:],
                                     mybir.ActivationFunctionType.Copy,
                                     scale=invn1[:])
                stt1 = ps1_p.tile([1, D], F32, space="PSUM", tag="stt")
                nc.tensor.transpose(out=stt1[:], in_=stats_g[:, 1:2],
                                    identity=ident_t[:D, :D])
                ex2_r = sm_p.tile([1, D], F32, tag="ex2_r")
                nc.scalar.activation(ex2_r[:], stt1[:],
                                     mybir.ActivationFunctionType.Copy,
                                     scale=invn1[:])
                mean_sq = sm_p.tile([1, D], F32, tag="mean_sq")
                nc.vector.tensor_tensor(out=mean_sq[:], in0=mean_r[:],
                                        in1=mean_r[:], op=mybir.AluOpType.mult)
                var = sm_p.tile([1, D], F32, tag="var")
                nc.vector.tensor_tensor(out=var[:], in0=ex2_r[:], in1=mean_sq[:],
                                        op=mybir.AluOpType.subtract)
                stdv = sm_p.tile([1, D], F32, tag="stdv")
                nc.scalar.activation(stdv[:], var[:],
                                     mybir.ActivationFunctionType.Sqrt,
                                     bias=eps1[:])
                rstd = sm_p.tile([1, D], F32, tag="rstd")
                nc.vector.reciprocal(rstd[:], stdv[:])
                g_sb = sm_p.tile([1, D], F32, tag="g_sb")
                nc.sync.dma_start(g_sb[:], gl[:])
                be_sb = sm_p.tile([1, D], F32, tag="be_sb")
                nc.sync.dma_start(be_sb[:], bel[:])
                scale_r = sm_p.tile([1, D], F32, tag="scale_r")
                nc.vector.tensor_tensor(out=scale_r[:], in0=g_sb[:], in1=rstd[:],
                                        op=mybir.AluOpType.mult)
                ms = sm_p.tile([1, D], F32, tag="ms")
                nc.vector.tensor_tensor(out=ms[:], in0=mean_r[:], in1=scale_r[:],
                                        op=mybir.AluOpType.mult)
                shift_r = sm_p.tile([1, D], F32, tag="shift_r")
                nc.vector.tensor_tensor(out=shift_r[:], in0=be_sb[:], in1=ms[:],
                                        op=mybir.AluOpType.subtract)
                both = sm_p.tile([1, 2 * D], F32, tag="both")
                nc.vector.tensor_copy(both[:, :D], scale_r[:])
                nc.vector.tensor_copy(both[:, D:], shift_r[:])
                bc_ps = ps1_p.tile([128, 2 * D], F32, space="PSUM", tag="stats")
                nc.tensor.matmul(bc_ps[:], lhsT=ones_row[:], rhs=both[:],
                                 start=True, stop=True)
                sc128 = sm_p.tile([128, 2 * D], F32, tag="sc128")
                nc.scalar.activation(sc128[:], bc_ps[:],
                                     mybir.ActivationFunctionType.Copy)
                nc.vector.tensor_tensor(
                    out=zs[:].rearrange("p (g e) -> p g e", e=D),
                    in0=zs[:].rearrange("p (g e) -> p g e", e=D),
                    in1=sc128[:, None, :D].to_broadcast([128, GD, D]),
                    op=mybir.AluOpType.mult)
                nc.vector.tensor_tensor(
                    out=zs[:].rearrange("p (g e) -> p g e", e=D),
                    in0=zs[:].rearrange("p (g e) -> p g e", e=D),
                    in1=sc128[:, None, D:].to_broadcast([128, GD, D]),
                    op=mybir.AluOpType.add)
                for p in range(2):
                    nc.vector.tensor_tensor(
                        out=zs[:].rearrange("p (g q e) -> p g q e", q=2, e=D)[:, :, p, :],
                        in0=zs[:].rearrange("p (g q e) -> p g q e", q=2, e=D)[:, :, p, :],
                        in1=dinv_t[p][:, :, None].to_broadcast([128, HC, D]),
                        op=mybir.AluOpType.mult)
                nc.scalar.activation(zs[:], zs[:],
                                     mybir.ActivationFunctionType.Relu)
                nc.sync.dma_start(
                    agin[:].rearrange("(g p) e -> p g e", p=128),
                    zs[:].rearrange("p (g e) -> p g e", e=D))
                nc.gpsimd.collective_compute(
                    "AllGather", mybir.AluOpType.bypass, ins=[agin[:]],
                    outs=[tnext[:]], replica_groups=RG)
            else:
                b3_sb = sm_p.tile([1, D], F32, tag="b3_sb")
                nc.sync.dma_start(b3_sb[:], b3[:])
                b3_ps = ps1_p.tile([128, D], F32, space="PSUM", tag="stats")
                nc.tensor.matmul(b3_ps[:], lhsT=ones_row[:], rhs=b3_sb[:],
                                 start=True, stop=True)
                b128 = sm_p.tile([128, D], F32, tag="b128")
                nc.scalar.activation(b128[:], b3_ps[:],
                                     mybir.ActivationFunctionType.Copy)
                nc.vector.tensor_tensor(
                    out=zs[:].rearrange("p (g e) -> p g e", e=D),
                    in0=zs[:].rearrange("p (g e) -> p g e", e=D),
                    in1=b128[:, None, :].to_broadcast([128, GD, D]),
                    op=mybir.AluOpType.add)
                nc.sync.dma_start(
                    y[:].rearrange("(g p) e -> p g e", p=128),
                    zs[:].rearrange("p (g e) -> p g e", e=D))

    nc.compile()
    return nc


def make_in_maps(meta, per_core, xt, params):
    in_maps = []
    for c in range(meta["n_cores"]):
        in_maps.append({
            "xt": xt, "gi": per_core[c]["gi"], "si": per_core[c]["si"],
            "dinv_e": per_core[c]["dinv_e"], "dinv_o": per_core[c]["dinv_o"],
            "W1": params["W1"], "W2": params["W2"], "W3": params["W3"],
            "g1": params["g1"].reshape(1, -1), "be1": params["be1"].reshape(1, -1),
            "g2": params["g2"].reshape(1, -1), "be2": params["be2"].reshape(1, -1),
            "b3": params["b3"].reshape(1, -1),
        })
    return in_maps


def collect_output(meta, results):
    N, D = meta["N"], meta["D"]
    SH = meta["SH"]
    out = np.zeros((N, D), np.float32)
    for c in range(meta["n_cores"]):
        out[c * SH:(c + 1) * SH] = results[c]["y"][:SH]
    return out


# ----------------------------------------------------------------------------
# Self-contained kernel entry point: takes FULL inputs, returns FULL output.
# ----------------------------------------------------------------------------
_CACHE = {}


def kernel(x, edge_index, W1, b1, g1, be1, W2, b2, g2, be2, W3, b3):
    x = np.asarray(x, dtype=np.float32)
    edge_index = np.asarray(edge_index)
    params = {
        "W1": np.asarray(W1, np.float32), "W2": np.asarray(W2, np.float32),
        "W3": np.asarray(W3, np.float32),
        "g1": np.asarray(g1, np.float32), "be1": np.asarray(be1, np.float32),
        "g2": np.asarray(g2, np.float32), "be2": np.asarray(be2, np.float32),
        "b3": np.asarray(b3, np.float32),
    }
    key = (x.shape, edge_index.shape, hash(edge_index.tobytes()))
    if key in _CACHE:
        meta, per_core, xt_shape, nc = _CACHE[key]
        # x may differ between calls with same edges; rebuild xt
        meta2, per_core2, xt = host_prep(x, edge_index, n_cores=8)
        per_core = per_core2
        meta = meta2
    else:
        meta, per_core, xt = host_prep(x, edge_index, n_cores=8)
        nc = build_kernel(meta)
        _CACHE[key] = (meta, per_core, xt.shape, nc)
    in_maps = make_in_maps(meta, per_core, xt, params)
    res = run_bass_kernel_spmd(nc, in_maps, list(range(meta["n_cores"])))
    out = collect_output(meta, res.results)
    # match reference output dtype (float32)
    return out
